# revision 3
# baseline (speedup 1.0000x reference)
"""Trainium2 Bass kernel for the Koopman-style encoder/recurrence model.

Sharding: pure data parallel — batch N=1024 split as 128 samples per core
across 8 NeuronCores; all parameters replicated (aux weights streamed from
HBM inside each core).

Per-core layout strategy:
  - Encoder runs weights-stationary: activations live as [feature_part,
    (t, n) moving]; xs is fed pre-transposed from the host. Results are
    transposed back per t-slice on the PE (identity-matmul transpose) so
    outputs leave SBUF as [n_part, (t, d)] with d contiguous in HBM.
  - The 128 aux MLPs run one net at a time with weights stationary;
    layer 3 flips to activation-stationary (lhsT = h2) so mu/omega land
    directly in [n_part, 2k] PSUM, giving a single evacuation for all nets.
  - The recurrence is replaced by its closed form
        y_t = exp(mu*dt*t) * R(omega*dt*t) @ y_0
    computed per t with ACT Sin/Exp (scale immediate = dt*t fused into the
    activation), so there is no serial dependency chain.
"""

import os
from contextlib import ExitStack

import numpy as np

import concourse.bacc as bacc
import concourse.bass as bass
import concourse.mybir as mybir
import concourse.tile as tile
from concourse.bass_utils import run_bass_kernel_spmd

AF = mybir.ActivationFunctionType
ALU = mybir.AluOpType
F32 = mybir.dt.float32

N, T, DIM, LDIM, ENC_H = 1024, 64, 32, 256, 256
NAUX, AUXH = 128, 128
DT = 0.01
NCORES = 8
NL = N // NCORES  # samples per core
D_OUT = DIM + LDIM  # 288
PI = float(np.pi)

_NC_CACHE = {}


def _body(ctx: ExitStack, tc: tile.TileContext, I, y_out, yp_out):
    nc = tc.nc

    cpool = ctx.enter_context(tc.tile_pool(name="consts", bufs=1))

    def load(name, eng=None):
        ap = I[name]
        t = cpool.tile(list(ap.shape), ap.dtype, name=f"s_{name}")
        (eng or nc.sync).dma_start(t[:], ap[:])
        return t

    sxsT = load("xsT")      # [32, T*NL] cols = t*NL + n
    sxs0 = load("xs0")      # [NL, 32]
    sW1 = load("eW1")       # [32, 256]
    sB1 = load("eB1")       # [128, 2]
    sW2 = load("eW2")       # [128, 512] cols = kh*256 + m
    sB2 = load("eB2")
    sW3 = load("eW3")       # [128, 512]
    sAW3 = load("aW3")      # [128, 2*NAUX]
    sAB1 = load("aB1")      # [128, NAUX] (h, k)
    sAB2 = load("aB2")
    sCWe = load("CWe")      # [128, 32]
    sCWo = load("CWo")
    sI = load("ident")      # [128, 128]

    # y[:, :, 0:32] = xs : straight DRAM->DRAM copy
    nc.sync.dma_start(
        y_out[:, :, 0:DIM],
        I["xsnat"].rearrange("n (t d) -> n t d", d=DIM),
    )

    persist = ctx.enter_context(tc.tile_pool(name="persist", bufs=1))
    MOs = persist.tile([NL, 2 * NAUX], F32, name="MOs")   # [n, 2k] mu/om interleaved
    MUt = persist.tile([NAUX, NL], F32, name="MUt")       # [k, n]
    OMt = persist.tile([NAUX, NL], F32, name="OMt")
    E0T = persist.tile([NL, LDIM], F32, name="E0T")       # [n, d] latent t=0
    E1t = persist.tile([NAUX, NL], F32, name="E1t")       # [k, n] even latents
    E2t = persist.tile([NAUX, NL], F32, name="E2t")

    # ---------------- aux MLPs (128 nets, weights streamed) ----------------
    with ExitStack() as actx:
        aw1p = actx.enter_context(tc.tile_pool(name="aw1", bufs=2))
        aw2p = actx.enter_context(tc.tile_pool(name="aw2", bufs=2))
        hp = actx.enter_context(tc.tile_pool(name="haux", bufs=3))
        ps_h1 = actx.enter_context(tc.tile_pool(name="psh1", bufs=2, space="PSUM"))
        ps_h2 = actx.enter_context(tc.tile_pool(name="psh2", bufs=2, space="PSUM"))
        ps_mo = actx.enter_context(tc.tile_pool(name="psmo", bufs=1, space="PSUM"))

        mo_ps = ps_mo.tile([NL, 2 * NAUX], F32, name="mo_ps")
        x0T = sxsT[:, 0:NL]  # [32, NL] == x0 transposed
        NB = 16  # nets per weight-DMA block
        for b in range(NAUX // NB):
            w1t = aw1p.tile([DIM, NB * AUXH], F32, tag="w1t")
            nc.sync.dma_start(w1t[:], I["aW1"][:, b * NB * AUXH:(b + 1) * NB * AUXH])
            w2t = aw2p.tile([AUXH, NB * AUXH], F32, tag="w2t")
            nc.sync.dma_start(w2t[:], I["aW2"][:, b * NB * AUXH:(b + 1) * NB * AUXH])
            for q in range(NB // 4):  # 4 nets per PSUM bank
                h1p = ps_h1.tile([128, 512], F32, tag="h1p")
                h2p = ps_h2.tile([128, 512], F32, tag="h2p")
                h1s = hp.tile([128, 512], F32, tag="h1s")
                h2s = hp.tile([128, 512], F32, tag="h2s")
                for j in range(4):
                    kl = q * 4 + j
                    nc.tensor.matmul(
                        h1p[:, j * 128:(j + 1) * 128],
                        w1t[:, kl * 128:(kl + 1) * 128],
                        x0T, start=True, stop=True,
                    )
                for j in range(4):
                    k = b * NB + q * 4 + j
                    nc.vector.tensor_scalar(
                        h1s[:, j * 128:(j + 1) * 128],
                        h1p[:, j * 128:(j + 1) * 128],
                        sAB1[:, k:k + 1], 0.0, ALU.add, ALU.max,
                    )
                for j in range(4):
                    kl = q * 4 + j
                    nc.tensor.matmul(
                        h2p[:, j * 128:(j + 1) * 128],
                        w2t[:, kl * 128:(kl + 1) * 128],
                        h1s[:, j * 128:(j + 1) * 128],
                        start=True, stop=True,
                    )
                for j in range(4):
                    k = b * NB + q * 4 + j
                    nc.scalar.activation(
                        h2s[:, j * 128:(j + 1) * 128],
                        h2p[:, j * 128:(j + 1) * 128],
                        AF.Relu, bias=sAB2[:, k:k + 1],
                    )
                for j in range(4):
                    k = b * NB + q * 4 + j
                    # layer 3, activation-stationary: out[n, 2] for net k
                    nc.tensor.matmul(
                        mo_ps[:, 2 * k:2 * k + 2],
                        h2s[:, j * 128:(j + 1) * 128],
                        sAW3[:, 2 * k:2 * k + 2],
                        start=True, stop=True,
                    )
        nc.scalar.copy(MOs[:], mo_ps[:])

    # ------------- encoder over all T (weights stationary) + y out -------------
    with ExitStack() as ectx:
        ep = ectx.enter_context(tc.tile_pool(name="enc", bufs=3))
        yp_pool = ectx.enter_context(tc.tile_pool(name="ytile", bufs=3))
        ps_e1 = ectx.enter_context(tc.tile_pool(name="pse1", bufs=1, space="PSUM"))
        ps_e2 = ectx.enter_context(tc.tile_pool(name="pse2", bufs=1, space="PSUM"))
        ps_e3 = ectx.enter_context(tc.tile_pool(name="pse3", bufs=2, space="PSUM"))
        ps_tp = ectx.enter_context(tc.tile_pool(name="pstp", bufs=2, space="PSUM"))
        ps_xp = ectx.enter_context(tc.tile_pool(name="psxp", bufs=2, space="PSUM"))

        RN = 256  # rows (t,n pairs) per chunk; 2 t-slices
        for c in range(T * NL // RN):
            col0 = c * RN
            e1p = ps_e1.tile([128, 512], F32, tag="e1p")
            for mb in range(2):
                nc.tensor.matmul(
                    e1p[:, mb * 256:(mb + 1) * 256],
                    sW1[:, mb * 128:(mb + 1) * 128],
                    sxsT[:, col0:col0 + RN], start=True, stop=True,
                )
            h1s = ep.tile([128, 512], F32, tag="eh1")
            for mb in range(2):
                nc.scalar.activation(
                    h1s[:, mb * 256:(mb + 1) * 256],
                    e1p[:, mb * 256:(mb + 1) * 256],
                    AF.Relu, bias=sB1[:, mb:mb + 1],
                )
            e2p = ps_e2.tile([128, 512], F32, tag="e2p")
            for mb in range(2):
                for kh in range(2):
                    nc.tensor.matmul(
                        e2p[:, mb * 256:(mb + 1) * 256],
                        sW2[:, kh * 256 + mb * 128:kh * 256 + (mb + 1) * 128],
                        h1s[:, kh * 256:(kh + 1) * 256],
                        start=(kh == 0), stop=(kh == 1),
                    )
            h2s = ep.tile([128, 512], F32, tag="eh2")
            for mb in range(2):
                nc.scalar.activation(
                    h2s[:, mb * 256:(mb + 1) * 256],
                    e2p[:, mb * 256:(mb + 1) * 256],
                    AF.Relu, bias=sB2[:, mb:mb + 1],
                )
            e3p = ps_e3.tile([128, 512], F32, tag="e3p")
            for mb in range(2):
                for kh in range(2):
                    nc.tensor.matmul(
                        e3p[:, mb * 256:(mb + 1) * 256],
                        sW3[:, kh * 256 + mb * 128:kh * 256 + (mb + 1) * 128],
                        h2s[:, kh * 256:(kh + 1) * 256],
                        start=(kh == 0), stop=(kh == 1),
                    )
            e3s = ep.tile([128, 512], F32, tag="e3s")
            for mb in range(2):
                nc.vector.tensor_copy(e3s[:, mb * 256:(mb + 1) * 256],
                                      e3p[:, mb * 256:(mb + 1) * 256])
            # transpose back to [n, d] per t-slice and assemble y latents
            yt = yp_pool.tile([NL, 2 * LDIM], F32, tag="yt")
            for tt in range(2):
                for db in range(2):
                    tpp = ps_tp.tile([128, 128], F32, tag="tp")
                    nc.tensor.transpose(
                        tpp[:],
                        e3s[:, db * 256 + tt * 128:db * 256 + tt * 128 + 128],
                        sI[:],
                    )
                    dst = yt[:, tt * LDIM + db * 128:tt * LDIM + (db + 1) * 128]
                    if db == 0:
                        nc.scalar.copy(dst, tpp[:])
                    else:
                        nc.vector.tensor_copy(dst, tpp[:])
                    if c == 0 and tt == 0:
                        dst0 = E0T[:, db * 128:(db + 1) * 128]
                        if db == 0:
                            nc.vector.tensor_copy(dst0, tpp[:])
                        else:
                            nc.scalar.copy(dst0, tpp[:])
            nc.sync.dma_start(
                y_out[:, 2 * c:2 * c + 2, DIM:D_OUT],
                yt.rearrange("n (t d) -> n t d", d=LDIM),
            )

        # mu/om -> [k, n]; e0 even/odd -> [k, n]
        mo3 = MOs.rearrange("n (k c) -> n k c", c=2)
        e03 = E0T.rearrange("n (k c) -> n k c", c=2)
        for src, dst in ((mo3[:, :, 0], MUt), (mo3[:, :, 1], OMt),
                         (e03[:, :, 0], E1t), (e03[:, :, 1], E2t)):
            tpp = ps_tp.tile([128, 128], F32, tag="tp")
            nc.tensor.transpose(tpp[:], src, sI[:])
            nc.scalar.copy(dst[:], tpp[:])

        # ---------------- latent trajectory, closed form ----------------
        trig = ectx.enter_context(tc.tile_pool(name="trig", bufs=3))
        tmp = ectx.enter_context(tc.tile_pool(name="ttmp", bufs=3))
        ypp = ectx.enter_context(tc.tile_pool(name="yptile", bufs=2))
        xpsb = ectx.enter_context(tc.tile_pool(name="xpsb", bufs=2))

        blocks = [(1, 17), (17, 33), (33, 49), (49, 64)]
        for (t0, t1) in blocks:
            nt = t1 - t0
            lead = 1 if t0 == 1 else 0  # first block also carries t=0
            ypt = ypp.tile([NL, (nt + lead) * D_OUT], F32, tag="ypt")
            if lead:
                # t=0 row: [x0, e0]
                nc.vector.tensor_copy(ypt[:, 0:DIM], sxs0[:])
                nc.gpsimd.tensor_copy(ypt[:, DIM:D_OUT], E0T[:])
            xps = xpsb.tile([32, nt * 128], F32, tag="xps")
            xpp = None
            for t in range(t0, t1):
                g = (t - t0) % 4
                ct = trig.tile([NAUX, NL], F32, tag="ct")
                st = trig.tile([NAUX, NL], F32, tag="st")
                ext = trig.tile([NAUX, NL], F32, tag="ext")
                nc.scalar.activation(ct[:], OMt[:], AF.Sin, scale=DT * t, bias=PI / 2)
                nc.scalar.activation(st[:], OMt[:], AF.Sin, scale=DT * t)
                nc.scalar.activation(ext[:], MUt[:], AF.Exp, scale=DT * t)
                p1 = tmp.tile([NAUX, NL], F32, tag="p1")
                p2 = tmp.tile([NAUX, NL], F32, tag="p2")
                p3 = tmp.tile([NAUX, NL], F32, tag="p3")
                p4 = tmp.tile([NAUX, NL], F32, tag="p4")
                nc.vector.tensor_mul(p1[:], E1t[:], ct[:])
                nc.vector.tensor_mul(p2[:], E2t[:], st[:])
                nc.gpsimd.tensor_mul(p3[:], E1t[:], st[:])
                nc.gpsimd.tensor_mul(p4[:], E2t[:], ct[:])
                d1 = tmp.tile([NAUX, NL], F32, tag="d1")
                d2 = tmp.tile([NAUX, NL], F32, tag="d2")
                nc.vector.tensor_sub(d1[:], p1[:], p2[:])
                nc.gpsimd.tensor_add(d2[:], p3[:], p4[:])
                y1 = tmp.tile([NAUX, NL], F32, tag="y1")
                y2 = tmp.tile([NAUX, NL], F32, tag="y2")
                nc.vector.tensor_mul(y1[:], d1[:], ext[:])
                nc.vector.tensor_mul(y2[:], d2[:], ext[:])
                # transpose to [n, k] and write latent outputs (stride-2)
                off = (t - t0 + lead) * D_OUT
                lat = ypt[:, off + DIM:off + D_OUT].rearrange("n (k c) -> n k c", c=2)
                y1p = ps_tp.tile([128, 128], F32, tag="tp")
                nc.tensor.transpose(y1p[:], y1[:], sI[:])
                nc.scalar.copy(lat[:, :, 0], y1p[:])
                y2p = ps_tp.tile([128, 128], F32, tag="tp")
                nc.tensor.transpose(y2p[:], y2[:], sI[:])
                nc.vector.tensor_copy(lat[:, :, 1], y2p[:])
                # x_pred accumulation: xp_t = Y1^T @ CWe + Y2^T @ CWo
                if g == 0:
                    xpp = ps_xp.tile([32, 512], F32, tag="xpp")
                nc.tensor.matmul(xpp[:, g * 128:(g + 1) * 128], sCWe[:], y1[:],
                                 start=True, stop=False)
                nc.tensor.matmul(xpp[:, g * 128:(g + 1) * 128], sCWo[:], y2[:],
                                 start=False, stop=True)
                if g == 3 or t == t1 - 1:
                    g0 = (t - t0) - g
                    nc.scalar.copy(xps[:, g0 * 128:(t - t0 + 1) * 128],
                                   xpp[:, 0:(g + 1) * 128])
            for t in range(t0, t1):
                off = (t - t0 + lead) * D_OUT
                xtp = ps_tp.tile([128, 128], F32, tag="tp")
                nc.tensor.transpose(
                    xtp[:, 0:32],
                    xps[:, (t - t0) * 128:(t - t0 + 1) * 128],
                    sI[0:32, 0:32],
                )
                nc.scalar.copy(ypt[:, off:off + DIM], xtp[:, 0:32])
            nc.sync.dma_start(
                yp_out[:, t0 - lead:t1, :],
                ypt.rearrange("n (t d) -> n t d", d=D_OUT),
            )


def build_nc():
    nc = bacc.Bacc("TRN2", target_bir_lowering=False, debug=False,
                   enable_asserts=False)
    # register pi/2 as a const AP so Sin-with-bias can phase-shift to cos
    for val in (PI / 2,):
        t = nc.alloc_sbuf_tensor(f"const-f32-{val}", [128, 1], F32)
        nc.gpsimd.memset(t.ap(), val)
        nc.const_aps.aps[(F32, float(val))] = t.ap()
    nc.all_engine_barrier()
    shapes = {
        "xsT": (DIM, T * NL),
        "xs0": (NL, DIM),
        "xsnat": (NL, T * DIM),
        "eW1": (DIM, ENC_H),
        "eB1": (128, 2),
        "eW2": (128, 2 * ENC_H),
        "eB2": (128, 2),
        "eW3": (128, 2 * LDIM),
        "aW1": (DIM, NAUX * AUXH),
        "aB1": (AUXH, NAUX),
        "aW2": (AUXH, NAUX * AUXH),
        "aB2": (AUXH, NAUX),
        "aW3": (AUXH, NAUX * 2),
        "CWe": (NAUX, DIM),
        "CWo": (NAUX, DIM),
        "ident": (128, 128),
    }
    I = {k: nc.dram_tensor(k, list(v), F32, kind="ExternalInput").ap()
         for k, v in shapes.items()}
    y_out = nc.dram_tensor("y_out", [NL, T, D_OUT], F32, kind="ExternalOutput").ap()
    yp_out = nc.dram_tensor("yp_out", [NL, T, D_OUT], F32, kind="ExternalOutput").ap()
    with tile.TileContext(nc) as tc:
        with ExitStack() as ctx:
            _body(ctx, tc, I, y_out, yp_out)
    nc.compile()
    return nc


def host_inputs(xs_c, inputs):
    """Per-core input map from that core's xs shard + full weights."""
    f = np.float32
    aW1 = np.asarray(inputs["aux_W1"], f)
    aW2 = np.asarray(inputs["aux_W2"], f)
    aW3 = np.asarray(inputs["aux_W3"], f)
    CW = np.asarray(inputs["C_W"], f)
    m = {
        "xsT": xs_c.transpose(2, 1, 0).reshape(DIM, T * NL),
        "xs0": xs_c[:, 0, :],
        "xsnat": xs_c.reshape(NL, T * DIM),
        "eW1": np.asarray(inputs["enc_W1"], f),
        "eB1": np.asarray(inputs["enc_b1"], f).reshape(2, 128).T,
        "eW2": np.asarray(inputs["enc_W2"], f).reshape(2, 128, 256)
                 .transpose(1, 0, 2).reshape(128, 512),
        "eB2": np.asarray(inputs["enc_b2"], f).reshape(2, 128).T,
        "eW3": np.asarray(inputs["enc_W3"], f).reshape(2, 128, 256)
                 .transpose(1, 0, 2).reshape(128, 512),
        "aW1": aW1.transpose(1, 0, 2).reshape(DIM, NAUX * AUXH),
        "aB1": np.asarray(inputs["aux_b1"], f).T,
        "aW2": aW2.transpose(1, 0, 2).reshape(AUXH, NAUX * AUXH),
        "aB2": np.asarray(inputs["aux_b2"], f).T,
        "aW3": aW3.transpose(1, 0, 2).reshape(AUXH, NAUX * 2),
        "CWe": CW[0::2, :],
        "CWo": CW[1::2, :],
        "ident": np.eye(128, dtype=f),
    }
    return {k: np.ascontiguousarray(v, f) for k, v in m.items()}


def kernel(**inputs):
    xs = np.asarray(inputs["xs"], np.float32)
    if "nc" not in _NC_CACHE:
        _NC_CACHE["nc"] = build_nc()
    nc = _NC_CACHE["nc"]
    in_maps = [host_inputs(xs[c * NL:(c + 1) * NL], inputs) for c in range(NCORES)]
    res = run_bass_kernel_spmd(nc, in_maps, list(range(NCORES)),
                               trace=bool(os.environ.get("BASS_TRACE_RUN")))
    _NC_CACHE["last_results"] = res
    y = np.concatenate([res.results[c]["y_out"] for c in range(NCORES)], axis=0)
    yp = np.concatenate([res.results[c]["yp_out"] for c in range(NCORES)], axis=0)
    return y, yp
